# revision 13
# baseline (speedup 1.0000x reference)
"""MoE layer (E=8 routed experts top-2 + 1 shared, SwiGLU, H=1024, I=4096)
on 8 Trainium2 NeuronCores.

Strategy: expert parallelism. Core e holds routed expert e's weights and
processes the tokens routed to it (host-side dispatch, capacity padded);
the shared expert is token-sharded 512 tokens/core. Matmuls run in
fp16 (full PE rate, ~5e-4 rel err); router + losses + combine run on host
(router is ~0.02% of total FLOPs).

Self-contained: shapes hardcoded for hidden_states [2, 2048, 1024].
"""

import numpy as np

import concourse.mybir as mybir
import concourse.tile as tile
from concourse import bacc
from concourse.bass_utils import run_bass_kernel_spmd

E = 8
K = 2
H = 1024
I = 4096
N_SHARED = 1
LB_W = 0.01
Z_W = 0.01
P = 128
HT = H // P      # 8 h-tiles
IT = I // P      # 32 i-tiles
S = 512          # shared-expert tokens per core (T=4096 / 8)

F32 = mybir.dt.float32
F16 = mybir.dt.float16
GMAX = 1280  # max token-group width resident in SBUF (hT fits)

# Stash of the last device-run results (exec_time_ns etc.) for test harnesses.
LAST_RESULTS = None

_NC_CACHE = {}


def _split(W, piece):
    """Split width W into (offset, width) pieces of <= piece."""
    out = []
    c0 = 0
    while c0 < W:
        w = min(piece, W - c0)
        out.append((c0, w))
        c0 += w
    return out


def _build(C, CW):
    """Build the SPMD per-core Bass program.

    C: routed DRAM capacity (128-aligned). CW: exact computed token width
    (CW <= C; trailing padding columns are neither computed nor read).

    Per batch (routed C tokens / shared 512 tokens):
      - wd resident in SBUF (fp16, 32 tiles [128, 1024])
      - per token-group (<= GMAX): stream wg/wu once; pass A builds
        hT[it] = silu(wg.T x) * (wu.T x) in fp16; pass B accumulates
        out[h-tile] = sum_it wd[it].T @ hT[it] over 4-bank PSUM groups.
    """
    nc = bacc.Bacc("TRN2", target_bir_lowering=False, debug=False)

    xt_r = nc.dram_tensor("xt_r", [HT, P, C], F16, kind="ExternalInput")
    xt_s = nc.dram_tensor("xt_s", [HT, P, S], F16, kind="ExternalInput")
    wg_r = nc.dram_tensor("wg_r", [IT, P, H], F16, kind="ExternalInput")
    wu_r = nc.dram_tensor("wu_r", [IT, P, H], F16, kind="ExternalInput")
    wd_r = nc.dram_tensor("wd_r", [IT, P, H], F16, kind="ExternalInput")
    wg_s = nc.dram_tensor("wg_s", [IT, P, H], F16, kind="ExternalInput")
    wu_s = nc.dram_tensor("wu_s", [IT, P, H], F16, kind="ExternalInput")
    wd_s = nc.dram_tensor("wd_s", [IT, P, H], F16, kind="ExternalInput")
    out_r = nc.dram_tensor("out_r", [HT, P, C], F32, kind="ExternalOutput")
    out_s = nc.dram_tensor("out_s", [HT, P, S], F32, kind="ExternalOutput")
    # token-partition layout output for a trailing remainder slice (<=128
    # tokens): [token, H]. Avoids small-N matmul floor in pass B.
    out_m = nc.dram_tensor("out_m", [P, H], F32, kind="ExternalOutput")

    silu = mybir.ActivationFunctionType.Silu
    GW = min(GMAX, max(CW, S))  # widest token group -> tile sizes

    with tile.TileContext(nc) as tc:
        with (
            tc.tile_pool(name="xp", bufs=1) as xp,
            tc.tile_pool(name="wp", bufs=3) as wp,
            tc.tile_pool(name="wdp", bufs=1) as wdp,
            tc.tile_pool(name="hp", bufs=1) as hp,
            tc.tile_pool(name="sp", bufs=2) as sp,
            tc.tile_pool(name="op", bufs=3) as op,
            tc.tile_pool(name="psA", bufs=2, space="PSUM") as psA,
            tc.tile_pool(name="psB", bufs=4, space="PSUM") as psB,
        ):
            # PE warm-up: a short matmul burst on a scratch tile spans the
            # initial DMA ramp and flips HAM to full clock before real work
            # arrives. Two PSUM banks so the burst isn't bank-serialized.
            warm = sp.tile([P, 512], F16, tag="warm", name="warm")
            nc.gpsimd.memset(warm[:], 0.0)
            psW1 = psA.tile([P, 512], F32, tag="G", name="psW1")
            psW2 = psA.tile([P, 512], F32, tag="U", name="psW2")
            for _w in range(10):
                nc.tensor.matmul(
                    [psW1, psW2][_w % 2][:], warm[:, :P], warm[:],
                    start=True, stop=True,
                )

            batches = [
                (xt_s, wg_s, wu_s, wd_s, out_s, S),
                (xt_r, wg_r, wu_r, wd_r, out_r, CW),
            ]
            for xt_d, wg_d, wu_d, wd_d, out_d, W in batches:
                wds = [None] * IT  # resident wd tiles, loaded during pass A
                for g0, gw in _split(W, GMAX):
                    # it=0 weights first so the first matmuls' inputs
                    # aren't queued behind all 8 token-tile DMAs
                    w0 = []
                    for wsrc, wtag in ((wg_d, "wg"), (wu_d, "wu")):
                        wt = wp.tile([P, H], F16, tag=wtag, name=f"{wtag}0")
                        nc.sync.dma_start(wt[:], wsrc[0])
                        w0.append(wt)

                    # token tiles for this group
                    xts = []
                    for ht in range(HT):
                        xt = xp.tile([P, GW], F16, tag=f"xt{ht}", name=f"xt{ht}")
                        nc.sync.dma_start(xt[:, :gw], xt_d[ht][:, g0 : g0 + gw])
                        xts.append(xt)

                    css = _split(gw, 512)

                    # --- pass A ---
                    hts = []
                    for it in range(IT):
                        if it == 0:
                            wgt, wut = w0
                        else:
                            wgt = wp.tile([P, H], F16, tag="wg", name="wgt")
                            nc.sync.dma_start(wgt[:], wg_d[it])
                            wut = wp.tile([P, H], F16, tag="wu", name="wut")
                            nc.sync.dma_start(wut[:], wu_d[it])
                        if wds[it] is None:  # first group: stream resident wd
                            wdt = wdp.tile([P, H], F16, tag=f"wd{it}", name=f"wd{it}")
                            nc.sync.dma_start(wdt[:], wd_d[it])
                            wds[it] = wdt
                        ht_t = hp.tile([P, GW], F16, tag=f"h{it}", name=f"h{it}")
                        for c0, cw in css:
                            psG = psA.tile([P, 512], F32, tag="G", name="psG")
                            psU = psA.tile([P, 512], F32, tag="U", name="psU")
                            for ht in range(HT):
                                nc.tensor.matmul(
                                    psG[:, :cw],
                                    wgt[:, ht * P : (ht + 1) * P],
                                    xts[ht][:, c0 : c0 + cw],
                                    start=(ht == 0),
                                    stop=(ht == HT - 1),
                                )
                            for ht in range(HT):
                                nc.tensor.matmul(
                                    psU[:, :cw],
                                    wut[:, ht * P : (ht + 1) * P],
                                    xts[ht][:, c0 : c0 + cw],
                                    start=(ht == 0),
                                    stop=(ht == HT - 1),
                                )
                            st = sp.tile([P, 512], F32, tag="st", name="st")
                            nc.scalar.activation(st[:, :cw], psG[:, :cw], silu)
                            nc.vector.tensor_mul(
                                ht_t[:, c0 : c0 + cw], st[:, :cw], psU[:, :cw]
                            )
                        hts.append(ht_t)

                    # --- pass B ---
                    for c0, cw in css:
                        if cw <= P and out_d is out_r:
                            # swapped: tokens stationary (M=cw), wd moving
                            # (N=512) -> 64 MMs at full rate instead of 256
                            # floor-bound small-N MMs. Output [token, H].
                            for hh in range(2):
                                psR = psB.tile([P, 512], F32, tag="O", name=f"psR{hh}")
                                for it in range(IT):
                                    nc.tensor.matmul(
                                        psR[:cw, :],
                                        hts[it][:, c0 : c0 + cw],
                                        wds[it][:, hh * 512 : (hh + 1) * 512],
                                        start=(it == 0),
                                        stop=(it == IT - 1),
                                    )
                                om = op.tile([P, 512], F32, tag="o", name="om")
                                nc.vector.tensor_copy(om[:cw, :], psR[:cw, :])
                                nc.sync.dma_start(
                                    out_m[:cw, hh * 512 : (hh + 1) * 512],
                                    om[:cw, :],
                                )
                            continue
                        for hh in range(2):
                            psOs = [
                                psB.tile([P, 512], F32, tag="O", name=f"psO{q}")
                                for q in range(4)
                            ]
                            for it in range(IT):
                                for hq in range(4):
                                    nc.tensor.matmul(
                                        psOs[hq][:, :cw],
                                        wds[it][:, hh * 512 + hq * P : hh * 512 + (hq + 1) * P],
                                        hts[it][:, c0 : c0 + cw],
                                        start=(it == 0),
                                        stop=(it == IT - 1),
                                    )
                            for hq in range(4):
                                ot = op.tile([P, 512], F32, tag="o", name="ot")
                                nc.vector.tensor_copy(ot[:, :cw], psOs[hq][:, :cw])
                                nc.sync.dma_start(
                                    out_d[hh * 4 + hq][:, g0 + c0 : g0 + c0 + cw],
                                    ot[:, :cw],
                                )
    nc.compile()
    return nc


def _route(x, router_w):
    """Host router in float64: probs, top-2 (jax tie semantics), renorm."""
    logits = x.astype(np.float64) @ router_w.astype(np.float64)
    m = logits.max(-1, keepdims=True)
    ex = np.exp(logits - m)
    p = ex / ex.sum(-1, keepdims=True)
    topi = np.argsort(-p, axis=-1, kind="stable")[:, :K]
    topw = np.take_along_axis(p, topi, -1)
    topw = topw / topw.sum(-1, keepdims=True)
    return logits, topi, topw


def kernel(hidden_states, router_w, wg, wu, wd, sg, su, sd):
    global LAST_RESULTS
    B, SEQ, Hd = hidden_states.shape
    T = B * SEQ
    x = np.ascontiguousarray(hidden_states.reshape(T, Hd))

    logits, topi, topw = _route(x, router_w)

    # Per-expert token lists + combine weights
    idxs, cws = [], []
    for e in range(E):
        mask = (topi == e).any(-1)
        idx = np.nonzero(mask)[0]
        w = topw[idx][topi[idx] == e]
        idxs.append(idx)
        cws.append(w)
    counts = np.array([len(i) for i in idxs])
    C = max(128, int(-(-counts.max() // 128) * 128))
    CW = C

    nc = _NC_CACHE.get((C, CW))
    if nc is None:
        nc = _NC_CACHE[(C, CW)] = _build(C, CW)

    def tile_w_in(w):  # [H, I] -> [IT, P(h within tile), HT, I-tile] flat [IT,P,H]
        return np.ascontiguousarray(
            w.reshape(HT, P, IT, P).transpose(2, 1, 0, 3).reshape(IT, P, H)
        ).astype(np.float16)

    def tile_w_down(w):  # [I, H] -> [IT, P(i), H]
        return np.ascontiguousarray(w.reshape(IT, P, H)).astype(np.float16)

    wg_s_t = tile_w_in(sg[0])
    wu_s_t = tile_w_in(su[0])
    wd_s_t = tile_w_down(sd[0])

    in_maps = []
    for e in range(E):
        xe = np.zeros((C, Hd), np.float32)
        xe[: counts[e]] = x[idxs[e]]
        xs = x[e * S : (e + 1) * S]
        in_maps.append(
            {
                "xt_r": np.ascontiguousarray(xe.T.reshape(HT, P, C)).astype(np.float16),
                "xt_s": np.ascontiguousarray(xs.T.reshape(HT, P, S)).astype(np.float16),
                "wg_r": tile_w_in(wg[e]),
                "wu_r": tile_w_in(wu[e]),
                "wd_r": tile_w_down(wd[e]),
                "wg_s": wg_s_t,
                "wu_s": wu_s_t,
                "wd_s": wd_s_t,
            }
        )

    res = run_bass_kernel_spmd(nc, in_maps, list(range(E)))
    LAST_RESULTS = res

    rem_c0 = (CW // 512) * 512 if 0 < CW % 512 <= 128 else None
    out = np.zeros((T, Hd), np.float64)
    for e in range(E):
        y_r = res.results[e]["out_r"].reshape(Hd, C).T.copy()  # [C, H]
        if rem_c0 is not None:
            y_r[rem_c0:CW] = res.results[e]["out_m"][: CW - rem_c0]
        out[idxs[e]] += cws[e][:, None] * y_r[: counts[e]].astype(np.float64)
        y_s = res.results[e]["out_s"].reshape(Hd, S).T  # [S, H]
        out[e * S : (e + 1) * S] += y_s.astype(np.float64) / N_SHARED

    # Losses (host, float64 -> float32)
    loads = np.concatenate([counts.astype(np.float64), [float(T)] * N_SHARED])
    loads_norm = loads / loads.sum()
    ideal = 1.0 / (E + N_SHARED)
    lb = ((loads_norm - ideal) ** 2).mean()
    z = (logits**2).sum(-1).mean()
    total_loss = np.float32(LB_W * lb + Z_W * z)

    return out.reshape(B, SEQ, Hd).astype(np.float32), total_loss


# revision 14
# speedup vs baseline: 1.0025x; 1.0025x over previous
"""MoE layer (E=8 routed experts top-2 + 1 shared, SwiGLU, H=1024, I=4096)
on 8 Trainium2 NeuronCores.

Strategy: expert parallelism. Core e holds routed expert e's weights and
processes the tokens routed to it (host-side dispatch, capacity padded);
the shared expert is token-sharded 512 tokens/core. Matmuls run in
fp16 (full PE rate, ~5e-4 rel err); router + losses + combine run on host
(router is ~0.02% of total FLOPs).

Self-contained: shapes hardcoded for hidden_states [2, 2048, 1024].
"""

import numpy as np

import concourse.mybir as mybir
import concourse.tile as tile
from concourse import bacc
from concourse.bass_utils import run_bass_kernel_spmd

E = 8
K = 2
H = 1024
I = 4096
N_SHARED = 1
LB_W = 0.01
Z_W = 0.01
P = 128
HT = H // P      # 8 h-tiles
IT = I // P      # 32 i-tiles
S = 512          # shared-expert tokens per core (T=4096 / 8)

F32 = mybir.dt.float32
F16 = mybir.dt.float16
GMAX = 1280  # max token-group width resident in SBUF (hT fits)

# Stash of the last device-run results (exec_time_ns etc.) for test harnesses.
LAST_RESULTS = None

_NC_CACHE = {}


def _split(W, piece):
    """Split width W into (offset, width) pieces of <= piece."""
    out = []
    c0 = 0
    while c0 < W:
        w = min(piece, W - c0)
        out.append((c0, w))
        c0 += w
    return out


def _build(C, CW):
    """Build the SPMD per-core Bass program.

    C: routed DRAM capacity (128-aligned). CW: exact computed token width
    (CW <= C; trailing padding columns are neither computed nor read).

    Per batch (routed C tokens / shared 512 tokens):
      - wd resident in SBUF (fp16, 32 tiles [128, 1024])
      - per token-group (<= GMAX): stream wg/wu once; pass A builds
        hT[it] = silu(wg.T x) * (wu.T x) in fp16; pass B accumulates
        out[h-tile] = sum_it wd[it].T @ hT[it] over 4-bank PSUM groups.
    """
    nc = bacc.Bacc("TRN2", target_bir_lowering=False, debug=False)

    xt_r = nc.dram_tensor("xt_r", [HT, P, C], F16, kind="ExternalInput")
    xt_s = nc.dram_tensor("xt_s", [HT, P, S], F16, kind="ExternalInput")
    wg_r = nc.dram_tensor("wg_r", [IT, P, H], F16, kind="ExternalInput")
    wu_r = nc.dram_tensor("wu_r", [IT, P, H], F16, kind="ExternalInput")
    wd_r = nc.dram_tensor("wd_r", [IT, P, H], F16, kind="ExternalInput")
    wg_s = nc.dram_tensor("wg_s", [IT, P, H], F16, kind="ExternalInput")
    wu_s = nc.dram_tensor("wu_s", [IT, P, H], F16, kind="ExternalInput")
    wd_s = nc.dram_tensor("wd_s", [IT, P, H], F16, kind="ExternalInput")
    out_r = nc.dram_tensor("out_r", [HT, P, C], F32, kind="ExternalOutput")
    out_s = nc.dram_tensor("out_s", [HT, P, S], F32, kind="ExternalOutput")
    # token-partition layout output for a trailing remainder slice (<=128
    # tokens): [token, H]. Avoids small-N matmul floor in pass B.
    out_m = nc.dram_tensor("out_m", [P, H], F32, kind="ExternalOutput")

    silu = mybir.ActivationFunctionType.Silu
    GW = min(GMAX, max(CW, S))  # widest token group -> tile sizes

    with tile.TileContext(nc) as tc:
        with (
            tc.tile_pool(name="xp", bufs=1) as xp,
            tc.tile_pool(name="wp", bufs=3) as wp,
            tc.tile_pool(name="wdp", bufs=1) as wdp,
            tc.tile_pool(name="hp", bufs=1) as hp,
            tc.tile_pool(name="sp", bufs=2) as sp,
            tc.tile_pool(name="op", bufs=3) as op,
            tc.tile_pool(name="psA", bufs=2, space="PSUM") as psA,
            tc.tile_pool(name="psB", bufs=4, space="PSUM") as psB,
        ):
            # PE warm-up: a short matmul burst on a scratch tile spans the
            # initial DMA ramp and flips HAM to full clock before real work
            # arrives. Two PSUM banks so the burst isn't bank-serialized.
            warm = sp.tile([P, 512], F16, tag="warm", name="warm")
            nc.gpsimd.memset(warm[:], 0.0)
            psW1 = psA.tile([P, 512], F32, tag="G", name="psW1")
            psW2 = psA.tile([P, 512], F32, tag="U", name="psW2")
            for _w in range(10):
                nc.tensor.matmul(
                    [psW1, psW2][_w % 2][:], warm[:, :P], warm[:],
                    start=True, stop=True,
                )

            batches = [
                (xt_r, wg_r, wu_r, wd_r, out_r, CW),
                (xt_s, wg_s, wu_s, wd_s, out_s, S),
            ]
            for xt_d, wg_d, wu_d, wd_d, out_d, W in batches:
                wds = [None] * IT  # resident wd tiles, loaded during pass A
                for g0, gw in _split(W, GMAX):
                    # it=0 weights first so the first matmuls' inputs
                    # aren't queued behind all 8 token-tile DMAs
                    w0 = []
                    for wsrc, wtag in ((wg_d, "wg"), (wu_d, "wu")):
                        wt = wp.tile([P, H], F16, tag=wtag, name=f"{wtag}0")
                        nc.sync.dma_start(wt[:], wsrc[0])
                        w0.append(wt)

                    # token tiles for this group
                    xts = []
                    for ht in range(HT):
                        xt = xp.tile([P, GW], F16, tag=f"xt{ht}", name=f"xt{ht}")
                        nc.sync.dma_start(xt[:, :gw], xt_d[ht][:, g0 : g0 + gw])
                        xts.append(xt)

                    css = _split(gw, 512)

                    # --- pass A ---
                    hts = []
                    for it in range(IT):
                        if it == 0:
                            wgt, wut = w0
                        else:
                            wgt = wp.tile([P, H], F16, tag="wg", name="wgt")
                            nc.sync.dma_start(wgt[:], wg_d[it])
                            wut = wp.tile([P, H], F16, tag="wu", name="wut")
                            nc.sync.dma_start(wut[:], wu_d[it])
                        if wds[it] is None:  # first group: stream resident wd
                            wdt = wdp.tile([P, H], F16, tag=f"wd{it}", name=f"wd{it}")
                            nc.sync.dma_start(wdt[:], wd_d[it])
                            wds[it] = wdt
                        ht_t = hp.tile([P, GW], F16, tag=f"h{it}", name=f"h{it}")
                        for c0, cw in css:
                            psG = psA.tile([P, 512], F32, tag="G", name="psG")
                            psU = psA.tile([P, 512], F32, tag="U", name="psU")
                            for ht in range(HT):
                                nc.tensor.matmul(
                                    psG[:, :cw],
                                    wgt[:, ht * P : (ht + 1) * P],
                                    xts[ht][:, c0 : c0 + cw],
                                    start=(ht == 0),
                                    stop=(ht == HT - 1),
                                )
                            for ht in range(HT):
                                nc.tensor.matmul(
                                    psU[:, :cw],
                                    wut[:, ht * P : (ht + 1) * P],
                                    xts[ht][:, c0 : c0 + cw],
                                    start=(ht == 0),
                                    stop=(ht == HT - 1),
                                )
                            st = sp.tile([P, 512], F32, tag="st", name="st")
                            nc.scalar.activation(st[:, :cw], psG[:, :cw], silu)
                            nc.vector.tensor_mul(
                                ht_t[:, c0 : c0 + cw], st[:, :cw], psU[:, :cw]
                            )
                        hts.append(ht_t)

                    # --- pass B ---
                    for c0, cw in css:
                        if cw <= P and out_d is out_r:
                            # swapped: tokens stationary (M=cw), wd moving
                            # (N=512) -> 64 MMs at full rate instead of 256
                            # floor-bound small-N MMs. Output [token, H].
                            for hh in range(2):
                                psR = psB.tile([P, 512], F32, tag="O", name=f"psR{hh}")
                                for it in range(IT):
                                    nc.tensor.matmul(
                                        psR[:cw, :],
                                        hts[it][:, c0 : c0 + cw],
                                        wds[it][:, hh * 512 : (hh + 1) * 512],
                                        start=(it == 0),
                                        stop=(it == IT - 1),
                                    )
                                om = op.tile([P, 512], F32, tag="o", name="om")
                                nc.vector.tensor_copy(om[:cw, :], psR[:cw, :])
                                nc.sync.dma_start(
                                    out_m[:cw, hh * 512 : (hh + 1) * 512],
                                    om[:cw, :],
                                )
                            continue
                        for hh in range(2):
                            psOs = [
                                psB.tile([P, 512], F32, tag="O", name=f"psO{q}")
                                for q in range(4)
                            ]
                            for it in range(IT):
                                for hq in range(4):
                                    nc.tensor.matmul(
                                        psOs[hq][:, :cw],
                                        wds[it][:, hh * 512 + hq * P : hh * 512 + (hq + 1) * P],
                                        hts[it][:, c0 : c0 + cw],
                                        start=(it == 0),
                                        stop=(it == IT - 1),
                                    )
                            for hq in range(4):
                                ot = op.tile([P, 512], F32, tag="o", name="ot")
                                nc.vector.tensor_copy(ot[:, :cw], psOs[hq][:, :cw])
                                nc.sync.dma_start(
                                    out_d[hh * 4 + hq][:, g0 + c0 : g0 + c0 + cw],
                                    ot[:, :cw],
                                )
    nc.compile()
    return nc


def _route(x, router_w):
    """Host router in float64: probs, top-2 (jax tie semantics), renorm."""
    logits = x.astype(np.float64) @ router_w.astype(np.float64)
    m = logits.max(-1, keepdims=True)
    ex = np.exp(logits - m)
    p = ex / ex.sum(-1, keepdims=True)
    topi = np.argsort(-p, axis=-1, kind="stable")[:, :K]
    topw = np.take_along_axis(p, topi, -1)
    topw = topw / topw.sum(-1, keepdims=True)
    return logits, topi, topw


def kernel(hidden_states, router_w, wg, wu, wd, sg, su, sd):
    global LAST_RESULTS
    B, SEQ, Hd = hidden_states.shape
    T = B * SEQ
    x = np.ascontiguousarray(hidden_states.reshape(T, Hd))

    logits, topi, topw = _route(x, router_w)

    # Per-expert token lists + combine weights
    idxs, cws = [], []
    for e in range(E):
        mask = (topi == e).any(-1)
        idx = np.nonzero(mask)[0]
        w = topw[idx][topi[idx] == e]
        idxs.append(idx)
        cws.append(w)
    counts = np.array([len(i) for i in idxs])
    C = max(128, int(-(-counts.max() // 128) * 128))
    CW = C

    nc = _NC_CACHE.get((C, CW))
    if nc is None:
        nc = _NC_CACHE[(C, CW)] = _build(C, CW)

    def tile_w_in(w):  # [H, I] -> [IT, P(h within tile), HT, I-tile] flat [IT,P,H]
        return np.ascontiguousarray(
            w.reshape(HT, P, IT, P).transpose(2, 1, 0, 3).reshape(IT, P, H)
        ).astype(np.float16)

    def tile_w_down(w):  # [I, H] -> [IT, P(i), H]
        return np.ascontiguousarray(w.reshape(IT, P, H)).astype(np.float16)

    wg_s_t = tile_w_in(sg[0])
    wu_s_t = tile_w_in(su[0])
    wd_s_t = tile_w_down(sd[0])

    in_maps = []
    for e in range(E):
        xe = np.zeros((C, Hd), np.float32)
        xe[: counts[e]] = x[idxs[e]]
        xs = x[e * S : (e + 1) * S]
        in_maps.append(
            {
                "xt_r": np.ascontiguousarray(xe.T.reshape(HT, P, C)).astype(np.float16),
                "xt_s": np.ascontiguousarray(xs.T.reshape(HT, P, S)).astype(np.float16),
                "wg_r": tile_w_in(wg[e]),
                "wu_r": tile_w_in(wu[e]),
                "wd_r": tile_w_down(wd[e]),
                "wg_s": wg_s_t,
                "wu_s": wu_s_t,
                "wd_s": wd_s_t,
            }
        )

    res = run_bass_kernel_spmd(nc, in_maps, list(range(E)))
    LAST_RESULTS = res

    rem_c0 = (CW // 512) * 512 if 0 < CW % 512 <= 128 else None
    out = np.zeros((T, Hd), np.float64)
    for e in range(E):
        y_r = res.results[e]["out_r"].reshape(Hd, C).T.copy()  # [C, H]
        if rem_c0 is not None:
            y_r[rem_c0:CW] = res.results[e]["out_m"][: CW - rem_c0]
        out[idxs[e]] += cws[e][:, None] * y_r[: counts[e]].astype(np.float64)
        y_s = res.results[e]["out_s"].reshape(Hd, S).T  # [S, H]
        out[e * S : (e + 1) * S] += y_s.astype(np.float64) / N_SHARED

    # Losses (host, float64 -> float32)
    loads = np.concatenate([counts.astype(np.float64), [float(T)] * N_SHARED])
    loads_norm = loads / loads.sum()
    ideal = 1.0 / (E + N_SHARED)
    lb = ((loads_norm - ideal) ** 2).mean()
    z = (logits**2).sum(-1).mean()
    total_loss = np.float32(LB_W * lb + Z_W * z)

    return out.reshape(B, SEQ, Hd).astype(np.float32), total_loss


# revision 15
# speedup vs baseline: 1.0058x; 1.0033x over previous
"""MoE layer (E=8 routed experts top-2 + 1 shared, SwiGLU, H=1024, I=4096)
on 8 Trainium2 NeuronCores.

Strategy: expert parallelism. Core e holds routed expert e's weights and
processes the tokens routed to it (host-side dispatch, capacity padded);
the shared expert is token-sharded 512 tokens/core. Matmuls run in
fp16 (full PE rate, ~5e-4 rel err); router + losses + combine run on host
(router is ~0.02% of total FLOPs).

Self-contained: shapes hardcoded for hidden_states [2, 2048, 1024].
"""

import numpy as np

import concourse.mybir as mybir
import concourse.tile as tile
from concourse import bacc
from concourse.bass_utils import run_bass_kernel_spmd

E = 8
K = 2
H = 1024
I = 4096
N_SHARED = 1
LB_W = 0.01
Z_W = 0.01
P = 128
HT = H // P      # 8 h-tiles
IT = I // P      # 32 i-tiles
S = 512          # shared-expert tokens per core (T=4096 / 8)

F32 = mybir.dt.float32
F16 = mybir.dt.float16
GMAX = 1280  # max token-group width resident in SBUF (hT fits)

# Stash of the last device-run results (exec_time_ns etc.) for test harnesses.
LAST_RESULTS = None

_NC_CACHE = {}


def _split(W, piece):
    """Split width W into (offset, width) pieces of <= piece."""
    out = []
    c0 = 0
    while c0 < W:
        w = min(piece, W - c0)
        out.append((c0, w))
        c0 += w
    return out


def _build(C, CW):
    """Build the SPMD per-core Bass program.

    C: routed DRAM capacity (128-aligned). CW: exact computed token width
    (CW <= C; trailing padding columns are neither computed nor read).

    Per batch (routed C tokens / shared 512 tokens):
      - wd resident in SBUF (fp16, 32 tiles [128, 1024])
      - per token-group (<= GMAX): stream wg/wu once; pass A builds
        hT[it] = silu(wg.T x) * (wu.T x) in fp16; pass B accumulates
        out[h-tile] = sum_it wd[it].T @ hT[it] over 4-bank PSUM groups.
    """
    nc = bacc.Bacc("TRN2", target_bir_lowering=False, debug=False)

    xt_r = nc.dram_tensor("xt_r", [HT, P, C], F16, kind="ExternalInput")
    xt_s = nc.dram_tensor("xt_s", [HT, P, S], F16, kind="ExternalInput")
    wg_r = nc.dram_tensor("wg_r", [IT, P, H], F16, kind="ExternalInput")
    wu_r = nc.dram_tensor("wu_r", [IT, P, H], F16, kind="ExternalInput")
    wd_r = nc.dram_tensor("wd_r", [IT, P, H], F16, kind="ExternalInput")
    wg_s = nc.dram_tensor("wg_s", [IT, P, H], F16, kind="ExternalInput")
    wu_s = nc.dram_tensor("wu_s", [IT, P, H], F16, kind="ExternalInput")
    wd_s = nc.dram_tensor("wd_s", [IT, P, H], F16, kind="ExternalInput")
    out_r = nc.dram_tensor("out_r", [HT, P, C], F32, kind="ExternalOutput")
    out_s = nc.dram_tensor("out_s", [HT, P, S], F32, kind="ExternalOutput")
    # token-partition layout output for a trailing remainder slice (<=128
    # tokens): [token, H]. Avoids small-N matmul floor in pass B.
    out_m = nc.dram_tensor("out_m", [P, H], F32, kind="ExternalOutput")

    silu = mybir.ActivationFunctionType.Silu
    GW = min(GMAX, max(CW, S))  # widest token group -> tile sizes

    with tile.TileContext(nc) as tc:
        with (
            tc.tile_pool(name="xp", bufs=1) as xp,
            tc.tile_pool(name="wp", bufs=4) as wp,
            tc.tile_pool(name="wdp", bufs=1) as wdp,
            tc.tile_pool(name="hp", bufs=1) as hp,
            tc.tile_pool(name="sp", bufs=2) as sp,
            tc.tile_pool(name="op", bufs=4) as op,
            tc.tile_pool(name="psA", bufs=2, space="PSUM") as psA,
            tc.tile_pool(name="psB", bufs=4, space="PSUM") as psB,
        ):
            # PE warm-up: a short matmul burst on a scratch tile spans the
            # initial DMA ramp and flips HAM to full clock before real work
            # arrives. Two PSUM banks so the burst isn't bank-serialized.
            warm = sp.tile([P, 512], F16, tag="warm", name="warm")
            nc.gpsimd.memset(warm[:], 0.0)
            psW1 = psA.tile([P, 512], F32, tag="G", name="psW1")
            psW2 = psA.tile([P, 512], F32, tag="U", name="psW2")
            for _w in range(10):
                nc.tensor.matmul(
                    [psW1, psW2][_w % 2][:], warm[:, :P], warm[:],
                    start=True, stop=True,
                )

            batches = [
                (xt_r, wg_r, wu_r, wd_r, out_r, CW),
                (xt_s, wg_s, wu_s, wd_s, out_s, S),
            ]
            for xt_d, wg_d, wu_d, wd_d, out_d, W in batches:
                wds = [None] * IT  # resident wd tiles, loaded during pass A
                for g0, gw in _split(W, GMAX):
                    # it=0 weights first so the first matmuls' inputs
                    # aren't queued behind all 8 token-tile DMAs
                    w0 = []
                    for wsrc, wtag in ((wg_d, "wg"), (wu_d, "wu")):
                        wt = wp.tile([P, H], F16, tag=wtag, name=f"{wtag}0")
                        nc.sync.dma_start(wt[:], wsrc[0])
                        w0.append(wt)

                    # token tiles for this group
                    xts = []
                    for ht in range(HT):
                        xt = xp.tile([P, GW], F16, tag=f"xt{ht}", name=f"xt{ht}")
                        nc.sync.dma_start(xt[:, :gw], xt_d[ht][:, g0 : g0 + gw])
                        xts.append(xt)

                    css = _split(gw, 512)

                    # --- pass A ---
                    hts = []
                    for it in range(IT):
                        if it == 0:
                            wgt, wut = w0
                        else:
                            wgt = wp.tile([P, H], F16, tag="wg", name="wgt")
                            nc.sync.dma_start(wgt[:], wg_d[it])
                            wut = wp.tile([P, H], F16, tag="wu", name="wut")
                            nc.sync.dma_start(wut[:], wu_d[it])
                        if wds[it] is None:  # first group: stream resident wd
                            wdt = wdp.tile([P, H], F16, tag=f"wd{it}", name=f"wd{it}")
                            nc.sync.dma_start(wdt[:], wd_d[it])
                            wds[it] = wdt
                        ht_t = hp.tile([P, GW], F16, tag=f"h{it}", name=f"h{it}")
                        for c0, cw in css:
                            psG = psA.tile([P, 512], F32, tag="G", name="psG")
                            psU = psA.tile([P, 512], F32, tag="U", name="psU")
                            for ht in range(HT):
                                nc.tensor.matmul(
                                    psG[:, :cw],
                                    wgt[:, ht * P : (ht + 1) * P],
                                    xts[ht][:, c0 : c0 + cw],
                                    start=(ht == 0),
                                    stop=(ht == HT - 1),
                                )
                            for ht in range(HT):
                                nc.tensor.matmul(
                                    psU[:, :cw],
                                    wut[:, ht * P : (ht + 1) * P],
                                    xts[ht][:, c0 : c0 + cw],
                                    start=(ht == 0),
                                    stop=(ht == HT - 1),
                                )
                            st = sp.tile([P, 512], F32, tag="st", name="st")
                            nc.scalar.activation(st[:, :cw], psG[:, :cw], silu)
                            nc.vector.tensor_mul(
                                ht_t[:, c0 : c0 + cw], st[:, :cw], psU[:, :cw]
                            )
                        hts.append(ht_t)

                    # --- pass B ---
                    for c0, cw in css:
                        if cw <= P and out_d is out_r:
                            # swapped: tokens stationary (M=cw), wd moving
                            # (N=512) -> 64 MMs at full rate instead of 256
                            # floor-bound small-N MMs. Output [token, H].
                            for hh in range(2):
                                psR = psB.tile([P, 512], F32, tag="O", name=f"psR{hh}")
                                for it in range(IT):
                                    nc.tensor.matmul(
                                        psR[:cw, :],
                                        hts[it][:, c0 : c0 + cw],
                                        wds[it][:, hh * 512 : (hh + 1) * 512],
                                        start=(it == 0),
                                        stop=(it == IT - 1),
                                    )
                                om = op.tile([P, 512], F32, tag="o", name="om")
                                nc.vector.tensor_copy(om[:cw, :], psR[:cw, :])
                                nc.sync.dma_start(
                                    out_m[:cw, hh * 512 : (hh + 1) * 512],
                                    om[:cw, :],
                                )
                            continue
                        for hh in range(2):
                            psOs = [
                                psB.tile([P, 512], F32, tag="O", name=f"psO{q}")
                                for q in range(4)
                            ]
                            for it in range(IT):
                                for hq in range(4):
                                    nc.tensor.matmul(
                                        psOs[hq][:, :cw],
                                        wds[it][:, hh * 512 + hq * P : hh * 512 + (hq + 1) * P],
                                        hts[it][:, c0 : c0 + cw],
                                        start=(it == 0),
                                        stop=(it == IT - 1),
                                    )
                            for hq in range(4):
                                ot = op.tile([P, 512], F32, tag="o", name="ot")
                                nc.vector.tensor_copy(ot[:, :cw], psOs[hq][:, :cw])
                                nc.sync.dma_start(
                                    out_d[hh * 4 + hq][:, g0 + c0 : g0 + c0 + cw],
                                    ot[:, :cw],
                                )
    nc.compile()
    return nc


def _route(x, router_w):
    """Host router in float64: probs, top-2 (jax tie semantics), renorm."""
    logits = x.astype(np.float64) @ router_w.astype(np.float64)
    m = logits.max(-1, keepdims=True)
    ex = np.exp(logits - m)
    p = ex / ex.sum(-1, keepdims=True)
    topi = np.argsort(-p, axis=-1, kind="stable")[:, :K]
    topw = np.take_along_axis(p, topi, -1)
    topw = topw / topw.sum(-1, keepdims=True)
    return logits, topi, topw


def kernel(hidden_states, router_w, wg, wu, wd, sg, su, sd):
    global LAST_RESULTS
    B, SEQ, Hd = hidden_states.shape
    T = B * SEQ
    x = np.ascontiguousarray(hidden_states.reshape(T, Hd))

    logits, topi, topw = _route(x, router_w)

    # Per-expert token lists + combine weights
    idxs, cws = [], []
    for e in range(E):
        mask = (topi == e).any(-1)
        idx = np.nonzero(mask)[0]
        w = topw[idx][topi[idx] == e]
        idxs.append(idx)
        cws.append(w)
    counts = np.array([len(i) for i in idxs])
    C = max(128, int(-(-counts.max() // 128) * 128))
    CW = C

    nc = _NC_CACHE.get((C, CW))
    if nc is None:
        nc = _NC_CACHE[(C, CW)] = _build(C, CW)

    def tile_w_in(w):  # [H, I] -> [IT, P(h within tile), HT, I-tile] flat [IT,P,H]
        return np.ascontiguousarray(
            w.reshape(HT, P, IT, P).transpose(2, 1, 0, 3).reshape(IT, P, H)
        ).astype(np.float16)

    def tile_w_down(w):  # [I, H] -> [IT, P(i), H]
        return np.ascontiguousarray(w.reshape(IT, P, H)).astype(np.float16)

    wg_s_t = tile_w_in(sg[0])
    wu_s_t = tile_w_in(su[0])
    wd_s_t = tile_w_down(sd[0])

    in_maps = []
    for e in range(E):
        xe = np.zeros((C, Hd), np.float32)
        xe[: counts[e]] = x[idxs[e]]
        xs = x[e * S : (e + 1) * S]
        in_maps.append(
            {
                "xt_r": np.ascontiguousarray(xe.T.reshape(HT, P, C)).astype(np.float16),
                "xt_s": np.ascontiguousarray(xs.T.reshape(HT, P, S)).astype(np.float16),
                "wg_r": tile_w_in(wg[e]),
                "wu_r": tile_w_in(wu[e]),
                "wd_r": tile_w_down(wd[e]),
                "wg_s": wg_s_t,
                "wu_s": wu_s_t,
                "wd_s": wd_s_t,
            }
        )

    res = run_bass_kernel_spmd(nc, in_maps, list(range(E)))
    LAST_RESULTS = res

    rem_c0 = (CW // 512) * 512 if 0 < CW % 512 <= 128 else None
    out = np.zeros((T, Hd), np.float64)
    for e in range(E):
        y_r = res.results[e]["out_r"].reshape(Hd, C).T.copy()  # [C, H]
        if rem_c0 is not None:
            y_r[rem_c0:CW] = res.results[e]["out_m"][: CW - rem_c0]
        out[idxs[e]] += cws[e][:, None] * y_r[: counts[e]].astype(np.float64)
        y_s = res.results[e]["out_s"].reshape(Hd, S).T  # [S, H]
        out[e * S : (e + 1) * S] += y_s.astype(np.float64) / N_SHARED

    # Losses (host, float64 -> float32)
    loads = np.concatenate([counts.astype(np.float64), [float(T)] * N_SHARED])
    loads_norm = loads / loads.sum()
    ideal = 1.0 / (E + N_SHARED)
    lb = ((loads_norm - ideal) ** 2).mean()
    z = (logits**2).sum(-1).mean()
    total_loss = np.float32(LB_W * lb + Z_W * z)

    return out.reshape(B, SEQ, Hd).astype(np.float32), total_loss
